# revision 7
# baseline (speedup 1.0000x reference)
"""TRN2 Bass kernel for nn_Attention (dense transformer block with softmax over
the HEAD axis).

Reference computation (B=2, S=2048, E=1024, H=16, D=64):
    qkv = x @ W_qkv + b_qkv ; q,k,v split, heads
    scores = q k^T / 8, causal-masked to -1e4
    attn = softmax over HEADS (dim=1)  -> masked positions get exactly 1/16
    y = (attn @ v) @ W_dense + b_dense ; also returns cached_kv = stack(k, v)

Sharding: 8 cores = 2 batches x 4 query-chunks of 512. Fully uniform SPMD
program; all per-core variation is in host-prepared input data:
  - keys PERMUTED per core (own chunk first) so "my chunk" is a static slice
  - causal gates (a in {0,1}) zero out fully-masked tiles' attn
  - fully-masked tiles' 1/16*sum(v) contribution via a sel-matrix matmul
  - the diagonal tile's intra-tile mask via a static affine_select (fill 1/16)

Precision: fp16 compute path (q/k/v/E/attn ~0.05% rounding), f32 PSUM
accumulation, separate float32r matmuls for the cached_kv output chunk.
"""
import os
import sys

sys.path.insert(0, "/opt/trn_rl_repo")
os.environ.setdefault("JAX_PLATFORMS", "axon,cpu")

import numpy as np
import ml_dtypes

import concourse.bacc as bacc
import concourse.mybir as mybir
import concourse.tile as tile
from concourse.bass_utils import run_bass_kernel_spmd

F32 = mybir.dt.float32
F32R = mybir.dt.float32r
F16 = mybir.dt.float16
AF = mybir.ActivationFunctionType
ALU = mybir.AluOpType

B, S, E = 2, 2048, 1024
H, D = 16, 64
NCORES = 8
CH = 512              # q-chunk per core
NT = S // 128         # 16 key tiles
NQT = CH // 128       # 4 q tiles per core

LAST_RESULTS = None   # for test harness introspection
_CACHED_NC = None


def _build_nc():
    nc = bacc.Bacc("TRN2", target_bir_lowering=False, debug=False,
                   num_devices=NCORES)

    # ---- I/O ----
    xt_d = nc.dram_tensor("xt", [128, 8, S], F16, kind="ExternalInput")
    wqkv_d = nc.dram_tensor("wqkv", [128, 8, 3 * E], F16, kind="ExternalInput")
    wd_d = nc.dram_tensor("wd", [128, 8, E], F16, kind="ExternalInput")
    xtc_d = nc.dram_tensor("xtc", [128, 8, CH], F32, kind="ExternalInput")
    wkvf_d = nc.dram_tensor("wkvf", [128, 8, 2 * E], F32, kind="ExternalInput")
    bqkv_col_d = nc.dram_tensor("bqkv_col", [128, 16], F32, kind="ExternalInput")
    bkv_row_d = nc.dram_tensor("bkv_row", [1, 2 * E], F32, kind="ExternalInput")
    bv_row16_d = nc.dram_tensor("bv_row16", [1, E], F16, kind="ExternalInput")
    bd_row_d = nc.dram_tensor("bd_row", [1, E], F16, kind="ExternalInput")
    onesr_d = nc.dram_tensor("onesr", [1, 128], F32, kind="ExternalInput")
    agate_d = nc.dram_tensor("agate", [128, NT, NQT], F32, kind="ExternalInput")
    sel_d = nc.dram_tensor("sel", [128, NT, NQT], F16, kind="ExternalInput")

    y_d = nc.dram_tensor("y_out", [CH, E], F32, kind="ExternalOutput")
    k_d = nc.dram_tensor("k_out", [CH, E], F32, kind="ExternalOutput")
    v_d = nc.dram_tensor("v_out", [CH, E], F32, kind="ExternalOutput")

    with tile.TileContext(nc) as tc:
        with (
            tc.tile_pool(name="big", bufs=1) as big,       # kT, vsb, qT, yT
            tc.tile_pool(name="xts", bufs=2) as xts,       # xt n-block stream
            tc.tile_pool(name="wks", bufs=1) as wks,       # k-col m-tiles (all 8)
            tc.tile_pool(name="wqs", bufs=2) as wqs,       # q-col m-tiles
            tc.tile_pool(name="wvs", bufs=1) as wvs,       # v-col tiles (2 eh)
            tc.tile_pool(name="wfs", bufs=2) as wfs,       # f32r W stream
            tc.tile_pool(name="xcs", bufs=2) as xcs,       # f32r xtc stream
            tc.tile_pool(name="epool", bufs=2) as epool,   # E tiles (+wd reuse)
            tc.tile_pool(name="apool", bufs=1) as apool,   # attn tiles
            tc.tile_pool(name="dpool", bufs=2) as dpool,   # softmax scratch
            tc.tile_pool(name="stage", bufs=2) as stage,   # psum->dram staging
            tc.tile_pool(name="small", bufs=1) as small,   # consts
            tc.tile_pool(name="brow", bufs=2) as brow,     # bias row stream
            tc.tile_pool(name="ps", bufs=4, space="PSUM") as ps_pool,
            tc.tile_pool(name="ps_y", bufs=1, space="PSUM") as ps_y,
        ):
            # ---- resident small inputs ----
            bqkv_col = small.tile([128, 16], F32)
            onesr = small.tile([1, 128], F32R)
            ones16 = small.tile([1, 128], F16)
            agate = small.tile([128, NT, NQT], F32)
            sel = small.tile([128, NT, NQT], F16)
            nc.sync.dma_start(bqkv_col[:], bqkv_col_d[:])
            nc.sync.dma_start(onesr[:], onesr_d[:].bitcast(F32R))
            nc.sync.dma_start(agate[:], agate_d[:])
            nc.sync.dma_start(sel[:], sel_d[:])
            nc.vector.memset(ones16[:], 1.0)

            kT = big.tile([128, 8, S], F16)       # kT[p, m, s] = k_perm[s, 128m+p]
            vsb = big.tile([128, NT, E], F16)     # v[p, st, e] = v_perm[128st+p, e]
            qT = big.tile([128, 8, CH], F16)      # qT[p, m, q] (q pre-scaled 1/8)
            yT = big.tile([128, 8, CH], F16)      # yT[p, pair, q]
            st_sb = big.tile([128, 8, NQT], F32)  # suffix sums

            # persistent W tiles (loaded once, stay in their rings)
            wk = [wks.tile([128, 8, 128], F16, name=f"wk{m}", tag=f"wk{m}")
                  for m in range(8)]
            for m in range(8):
                nc.sync.dma_start(wk[m][:], wqkv_d[:, :, E + 128 * m:E + 128 * (m + 1)])
            wv = [wvs.tile([128, 8, 512], F16, name=f"wv{eh}", tag=f"wv{eh}")
                  for eh in range(2)]
            for eh in range(2):
                nc.sync.dma_start(
                    wv[eh][:], wqkv_d[:, :, 2 * E + 512 * eh:2 * E + 512 * (eh + 1)])

            # ============ stage A: qkv projections (fp16) ============
            for n in range(4):
                xt_n = xts.tile([128, 8, 512], F16, tag="xtn")
                nc.sync.dma_start(xt_n[:], xt_d[:, :, 512 * n:512 * (n + 1)])
                # kT m-tiles for this s-block
                for m in range(8):
                    ps = ps_pool.tile([128, 512], F32, tag="mm")
                    for ct in range(8):
                        nc.tensor.matmul(ps[:], wk[m][:, ct, :], xt_n[:, ct, :],
                                         start=(ct == 0), stop=(ct == 7))
                    nc.scalar.activation(kT[:, m, 512 * n:512 * (n + 1)], ps[:],
                                         AF.Identity, bias=bqkv_col[:, 8 + m:9 + m])
                # qT (q-chunk = first block of permuted keys)
                if n == 0:
                    for m in range(8):
                        wq = wqs.tile([128, 8, 128], F16, tag="wq")
                        nc.sync.dma_start(
                            wq[:], wqkv_d[:, :, 128 * m:128 * (m + 1)])
                        ps = ps_pool.tile([128, 512], F32, tag="mm")
                        for ct in range(8):
                            nc.tensor.matmul(ps[:], wq[:, ct, :], xt_n[:, ct, :],
                                             start=(ct == 0), stop=(ct == 7))
                        nc.scalar.activation(qT[:, m, :], ps[:], AF.Identity,
                                             bias=bqkv_col[:, m:m + 1])
                # v natural rows in this block
                for sl in range(4):
                    st = 4 * n + sl
                    for eh in range(2):
                        ps = ps_pool.tile([128, 512], F32, tag="mm")
                        for ct in range(8):
                            nc.tensor.matmul(
                                ps[:], xt_n[:, ct, 128 * sl:128 * (sl + 1)],
                                wv[eh][:, ct, :], start=(ct == 0), stop=False)
                        bv = brow.tile([1, 512], F16, tag="brow16")
                        nc.sync.dma_start(bv[:],
                                          bv_row16_d[:, 512 * eh:512 * (eh + 1)])
                        nc.tensor.matmul(ps[:], ones16[:], bv[:],
                                         start=False, stop=True)
                        nc.vector.tensor_copy(vsb[:, st, 512 * eh:512 * (eh + 1)],
                                              ps[:])

            # ---- f32r chunk path: exact k_out / v_out (cached_kv) ----
            for part in range(2):        # 0 = k cols, 1 = v cols
                for eh in range(2):
                    col0 = E * part + 512 * eh
                    pss = []
                    for st in range(NQT):
                        pss.append(ps_pool.tile([128, 512], F32, tag="mm",
                                                name=f"cps{part}{eh}{st}"))
                    for ct in range(8):
                        w = wfs.tile([128, 512], F32R, tag="wkv")
                        nc.sync.dma_start(
                            w[:], wkvf_d[:, ct, col0:col0 + 512].bitcast(F32R))
                        xc = xcs.tile([128, 512], F32R, tag="xtc")
                        nc.sync.dma_start(xc[:], xtc_d[:, ct, :].bitcast(F32R))
                        for st in range(NQT):
                            nc.tensor.matmul(
                                pss[st][:], xc[:, 128 * st:128 * (st + 1)],
                                w[:], start=(ct == 0), stop=False)
                    bkv = brow.tile([1, 512], F32R, tag="browr")
                    nc.sync.dma_start(
                        bkv[:], bkv_row_d[:, col0:col0 + 512].bitcast(F32R))
                    for st in range(NQT):
                        nc.tensor.matmul(pss[st][:], onesr[:], bkv[:],
                                         start=False, stop=True)
                        out_sb = stage.tile([128, 512], F32, tag="stg")
                        nc.vector.tensor_copy(out_sb[:], pss[st][:])
                        dst = k_d if part == 0 else v_d
                        nc.sync.dma_start(
                            dst[128 * st:128 * (st + 1), 512 * eh:512 * (eh + 1)],
                            out_sb[:])

            # ============ stage B: attention ============
            # suffix-sum matmuls first (uses the "mm" psum ring, then released)
            st_ps = ps_pool.tile([128, 8, NQT], F32, tag="mm")
            for dt in range(8):
                for kt in range(NT):
                    nc.tensor.matmul(
                        st_ps[:, dt, :], vsb[:, kt, 128 * dt:128 * (dt + 1)],
                        sel[:, kt, :], start=(dt == 0 and kt == 0),
                        stop=(kt == NT - 1))
            nc.vector.tensor_copy(st_sb[:], st_ps[:])

            # two sequential q-halves of 256 (bounds PSUM: yps 8KB + mm ring 8KB)
            for hf in range(2):
                q0 = 256 * hf
                yps = ps_y.tile([128, 8, 256], F32, tag="y", name=f"yps{hf}")
                for kt in range(NT):
                    e_t = epool.tile([128, H, 256], F16, tag="E")
                    for h in range(H):
                        po = 64 * (h % 2)
                        sc = ps_pool.tile([128, 256], F32, tag="mm")
                        nc.tensor.matmul(
                            sc[:], kT[po:po + 64, h // 2, 128 * kt:128 * (kt + 1)],
                            qT[po:po + 64, h // 2, q0:q0 + 256],
                            start=True, stop=True)
                        nc.scalar.activation(e_t[:, h, :], sc[:], AF.Exp)
                    # denominator tree (fp16, DVE 2x)
                    t8 = [dpool.tile([128, 256], F16, tag=f"d8_{i%4}",
                                     name=f"t8_{i}") for i in range(8)]
                    for i in range(8):
                        nc.vector.tensor_add(t8[i][:], e_t[:, 2 * i, :],
                                             e_t[:, 2 * i + 1, :])
                    t4 = [dpool.tile([128, 256], F16, tag=f"d4_{i%2}",
                                     name=f"t4_{i}") for i in range(4)]
                    for i in range(4):
                        nc.vector.tensor_add(t4[i][:], t8[2 * i][:], t8[2 * i + 1][:])
                    t2 = [dpool.tile([128, 256], F16, tag="d2", name=f"t2_{i}")
                          for i in range(2)]
                    for i in range(2):
                        nc.vector.tensor_add(t2[i][:], t4[2 * i][:], t4[2 * i + 1][:])
                    dd = dpool.tile([128, 256], F16, tag="dd")
                    nc.vector.tensor_add(dd[:], t2[0][:], t2[1][:])
                    # R = exp(-ln(D)); gated R' per q-tile
                    lnd = dpool.tile([128, 256], F32, tag="lnd")
                    nc.scalar.activation(lnd[:], dd[:], AF.Ln)
                    rr = dpool.tile([128, 256], F16, tag="rr")
                    nc.scalar.activation(rr[:], lnd[:], AF.Exp, scale=-1.0)
                    rg = dpool.tile([128, 256], F16, tag="rg")
                    for jj in range(2):
                        j = 2 * hf + jj
                        nc.vector.tensor_scalar_mul(
                            rg[:, 128 * jj:128 * (jj + 1)],
                            rr[:, 128 * jj:128 * (jj + 1)],
                            agate[:, kt, j:j + 1])
                    att = apool.tile([128, H, 256], F16, tag="att")
                    for h in range(H):
                        nc.vector.tensor_mul(att[:, h, :], e_t[:, h, :], rg[:])
                    for h in range(H):
                        po = 64 * (h % 2)
                        nc.tensor.matmul(
                            yps[po:po + 64, h // 2, :],
                            vsb[:, kt, 64 * h:64 * (h + 1)],
                            att[:, h, :],
                            start=(kt == 0 and h % 4 < 2),
                            stop=(kt == NT - 1))
                    # diagonal tile: exact softmax w/ intra-tile causal fill 1/16
                    if kt < NQT and kt // 2 == hf:
                        j = kt
                        jj = j % 2
                        datt = dpool.tile([128, H, 128], F16, tag="datt", bufs=1)
                        for h in range(H):
                            nc.vector.tensor_mul(
                                datt[:, h, :], e_t[:, h, 128 * jj:128 * (jj + 1)],
                                rr[:, 128 * jj:128 * (jj + 1)])
                            nc.gpsimd.affine_select(
                                out=datt[:, h, :], in_=datt[:, h, :],
                                compare_op=ALU.is_ge, fill=0.0625, base=0,
                                pattern=[[1, 128]], channel_multiplier=-1)
                        for h in range(H):
                            po = 64 * (h % 2)
                            nc.tensor.matmul(
                                yps[po:po + 64, h // 2,
                                    128 * jj:128 * (jj + 1)],
                                vsb[:, kt, 64 * h:64 * (h + 1)],
                                datt[:, h, :], start=False, stop=False)
                # finalize this half: yT = yps + suffix (per-partition scalar)
                for pair in range(8):
                    for jj in range(2):
                        j = 2 * hf + jj
                        nc.vector.tensor_scalar_add(
                            yT[:, pair, 128 * j:128 * (j + 1)],
                            yps[:, pair, 128 * jj:128 * (jj + 1)],
                            st_sb[:, pair, j:j + 1])

            # ============ stage C: dense ============
            for eh in range(2):
                wd_t = epool.tile([128, 8, 512], F16, tag="E", name=f"wd{eh}")
                nc.sync.dma_start(wd_t[:], wd_d[:, :, 512 * eh:512 * (eh + 1)])
                bd = brow.tile([1, 512], F16, tag="brow16")
                nc.sync.dma_start(bd[:], bd_row_d[:, 512 * eh:512 * (eh + 1)])
                for qt in range(NQT):
                    ps = ps_pool.tile([128, 512], F32, tag="mm")
                    for ct in range(8):
                        nc.tensor.matmul(
                            ps[:], yT[:, ct, 128 * qt:128 * (qt + 1)],
                            wd_t[:, ct, :], start=(ct == 0), stop=False)
                    nc.tensor.matmul(ps[:], ones16[:], bd[:],
                                     start=False, stop=True)
                    y_sb = stage.tile([128, 512], F32, tag="stg")
                    nc.vector.tensor_copy(y_sb[:], ps[:])
                    nc.sync.dma_start(
                        y_d[128 * qt:128 * (qt + 1), 512 * eh:512 * (eh + 1)],
                        y_sb[:])

    nc.compile()
    return nc


def _prep_core_inputs(x, W_qkv, b_qkv, W_dense, b_dense, core):
    b, c = core % 2, core // 2
    own = np.arange(CH * c, CH * (c + 1))
    rest = np.concatenate([np.arange(0, CH * c), np.arange(CH * (c + 1), S)])
    perm = np.concatenate([own, rest])
    orig_tile = np.concatenate([np.arange(4 * c, 4 * c + 4),
                                np.array([t for t in range(NT)
                                          if not (4 * c <= t < 4 * c + 4)])])

    xb = x[b]                                    # [S, E]
    xt = np.ascontiguousarray(xb[perm].T)        # [E, S]
    xt16 = xt.reshape(8, 128, S).transpose(1, 0, 2).astype(np.float16)

    wq = (W_qkv[:, :E] / 8.0)
    wqkv_mod = np.concatenate([wq, W_qkv[:, E:]], axis=1)
    wqkv16 = wqkv_mod.reshape(8, 128, 3 * E).transpose(1, 0, 2).astype(np.float16)
    wd16 = W_dense.reshape(8, 128, E).transpose(1, 0, 2).astype(np.float16)

    xtc = np.ascontiguousarray(xb[own].T)        # [E, CH] f32
    xtc = xtc.reshape(8, 128, CH).transpose(1, 0, 2).astype(np.float32)
    wkvf = W_qkv[:, E:].reshape(8, 128, 2 * E).transpose(1, 0, 2).astype(np.float32)

    bq = b_qkv[:E] / 8.0
    bqkv_col = np.concatenate([bq, b_qkv[E:2 * E]]).reshape(16, 128).T.astype(np.float32)
    bkv_row = b_qkv[E:].reshape(1, 2 * E).astype(np.float32)
    bv_row16 = b_qkv[2 * E:].reshape(1, E).astype(np.float16)
    bd_row = b_dense.reshape(1, E).astype(np.float16)
    onesr = np.ones((1, 128), np.float32)

    qt_g = 4 * c + np.arange(NQT)                # global q-tile indices
    ot = orig_tile[:, None]                      # [NT, 1]
    a = (ot < qt_g[None, :]).astype(np.float32)  # full-attention gate
    s = np.where(ot > qt_g[None, :], 1.0 / 16.0, 0.0).astype(np.float32)
    agate = np.broadcast_to(a, (128, NT, NQT)).astype(np.float32)
    sel = np.broadcast_to(s, (128, NT, NQT)).astype(np.float16)

    return dict(xt=np.ascontiguousarray(xt16), wqkv=np.ascontiguousarray(wqkv16),
                wd=np.ascontiguousarray(wd16), xtc=np.ascontiguousarray(xtc),
                wkvf=np.ascontiguousarray(wkvf), bqkv_col=bqkv_col,
                bkv_row=bkv_row, bv_row16=bv_row16, bd_row=bd_row, onesr=onesr,
                agate=np.ascontiguousarray(agate), sel=np.ascontiguousarray(sel))


def kernel(x, W_qkv, b_qkv, W_dense, b_dense):
    global LAST_RESULTS, _CACHED_NC
    x = np.asarray(x, np.float32)
    W_qkv = np.asarray(W_qkv, np.float32)
    b_qkv = np.asarray(b_qkv, np.float32)
    W_dense = np.asarray(W_dense, np.float32)
    b_dense = np.asarray(b_dense, np.float32)

    if _CACHED_NC is None:
        _CACHED_NC = _build_nc()
    nc = _CACHED_NC

    in_maps = [_prep_core_inputs(x, W_qkv, b_qkv, W_dense, b_dense, core)
               for core in range(NCORES)]
    res = run_bass_kernel_spmd(nc, in_maps, core_ids=list(range(NCORES)))
    LAST_RESULTS = res

    y = np.zeros((B, S, E), np.float32)
    kv = np.zeros((B, 2, H, S, D), np.float32)
    for core in range(NCORES):
        b, c = core % 2, core // 2
        r = res.results[core]
        y[b, CH * c:CH * (c + 1)] = r["y_out"]
        kv[b, 0, :, CH * c:CH * (c + 1), :] = \
            r["k_out"].reshape(CH, H, D).transpose(1, 0, 2)
        kv[b, 1, :, CH * c:CH * (c + 1), :] = \
            r["v_out"].reshape(CH, H, D).transpose(1, 0, 2)
    return y, kv


# revision 9
# speedup vs baseline: 1.4746x; 1.4746x over previous
"""TRN2 Bass kernel for nn_Attention (dense transformer block with softmax over
the HEAD axis).

Reference computation (B=2, S=2048, E=1024, H=16, D=64):
    qkv = x @ W_qkv + b_qkv ; q,k,v split, heads
    scores = q k^T / 8, causal-masked to -1e4
    attn = softmax over HEADS (dim=1)  -> masked positions get exactly 1/16
    y = (attn @ v) @ W_dense + b_dense ; also returns cached_kv = stack(k, v)

Sharding: 8 cores = 2 batches x 4 query-chunks of 512. Fully uniform SPMD
program; all per-core variation is in host-prepared input data:
  - keys PERMUTED per core (own chunk first) so "my chunk" is a static slice
  - causal gates (a in {0,1}) zero out fully-masked tiles' attn
  - fully-masked tiles' 1/16*sum(v) contribution via a sel-matrix matmul
  - the diagonal tile's intra-tile mask via a static affine_select (fill 1/16)

Precision: fp16 compute path (q/k/v/E/attn ~0.05% rounding), f32 PSUM
accumulation, separate float32r matmuls for the cached_kv output chunk.
"""
import os
import sys

sys.path.insert(0, "/opt/trn_rl_repo")
os.environ.setdefault("JAX_PLATFORMS", "axon,cpu")

import numpy as np
import ml_dtypes

import concourse.bacc as bacc
import concourse.mybir as mybir
import concourse.tile as tile
from concourse.bass_utils import run_bass_kernel_spmd

F32 = mybir.dt.float32
F32R = mybir.dt.float32r
F16 = mybir.dt.float16
AF = mybir.ActivationFunctionType
ALU = mybir.AluOpType

B, S, E = 2, 2048, 1024
H, D = 16, 64
NCORES = 8
CH = 512              # q-chunk per core
NT = S // 128         # 16 key tiles
NQT = CH // 128       # 4 q tiles per core

LAST_RESULTS = None   # for test harness introspection
_CACHED_NC = None


def _build_nc():
    nc = bacc.Bacc("TRN2", target_bir_lowering=False, debug=False,
                   num_devices=NCORES)

    # ---- I/O ----
    xt_d = nc.dram_tensor("xt", [128, 8, S], F16, kind="ExternalInput")
    wqkv_d = nc.dram_tensor("wqkv", [128, 8, 3 * E], F16, kind="ExternalInput")
    wd_d = nc.dram_tensor("wd", [128, 8, E], F16, kind="ExternalInput")
    bqkv_col_d = nc.dram_tensor("bqkv_col", [128, 16], F32, kind="ExternalInput")
    bv_row16_d = nc.dram_tensor("bv_row16", [1, E], F16, kind="ExternalInput")
    bd_row_d = nc.dram_tensor("bd_row", [1, E], F16, kind="ExternalInput")
    agate_d = nc.dram_tensor("agate", [128, NT, NQT], F32, kind="ExternalInput")
    sel_d = nc.dram_tensor("sel", [128, NT, NQT], F16, kind="ExternalInput")

    y_d = nc.dram_tensor("y_out", [CH, E], F32, kind="ExternalOutput")
    k_d = nc.dram_tensor("k_out", [128, 8, CH], F16, kind="ExternalOutput")
    v_d = nc.dram_tensor("v_out", [128, NQT, E], F16, kind="ExternalOutput")

    with tile.TileContext(nc) as tc:
        with (
            tc.tile_pool(name="big", bufs=1) as big,       # kT, vsb, qT, yT
            tc.tile_pool(name="xts", bufs=2) as xts,       # xt n-block stream
            tc.tile_pool(name="wks", bufs=1) as wks,       # k-col m-tiles (all 8)
            tc.tile_pool(name="wqs", bufs=2) as wqs,       # q-col m-tiles
            tc.tile_pool(name="wvs", bufs=1) as wvs,       # v-col tiles (2 eh)
            tc.tile_pool(name="epool", bufs=2) as epool,   # E tiles (+wd reuse)
            tc.tile_pool(name="apool", bufs=2) as apool,   # attn tiles
            tc.tile_pool(name="dpool", bufs=2) as dpool,   # softmax scratch
            tc.tile_pool(name="stage", bufs=2) as stage,   # psum->dram staging
            tc.tile_pool(name="small", bufs=1) as small,   # consts
            tc.tile_pool(name="brow", bufs=2) as brow,     # bias row stream
            tc.tile_pool(name="ps", bufs=4, space="PSUM") as ps_pool,
            tc.tile_pool(name="ps_y", bufs=1, space="PSUM") as ps_y,
        ):
            # ---- resident small inputs ----
            bqkv_col = small.tile([128, 16], F32)
            ones16 = small.tile([1, 128], F16)
            agate = small.tile([128, NT, NQT], F32)
            sel = small.tile([128, NT, NQT], F16)
            nc.sync.dma_start(bqkv_col[:], bqkv_col_d[:])
            nc.sync.dma_start(agate[:], agate_d[:])
            nc.sync.dma_start(sel[:], sel_d[:])
            nc.vector.memset(ones16[:], 1.0)

            kT = big.tile([128, 8, S], F16)       # kT[p, m, s] = k_perm[s, 128m+p]
            vsb = big.tile([128, NT, E], F16)     # v[p, st, e] = v_perm[128st+p, e]
            qT = big.tile([128, 8, CH], F16)      # qT[p, m, q] (q pre-scaled 1/8)
            yT = big.tile([128, 8, CH], F16)      # yT[p, pair, q]
            st_sb = big.tile([128, 8, NQT], F32)  # suffix sums

            # persistent W tiles (loaded once, stay in their rings)
            wk = [wks.tile([128, 8, 128], F16, name=f"wk{m}", tag=f"wk{m}")
                  for m in range(8)]
            for m in range(8):
                nc.sync.dma_start(wk[m][:], wqkv_d[:, :, E + 128 * m:E + 128 * (m + 1)])
            wv = [wvs.tile([128, 8, 512], F16, name=f"wv{eh}", tag=f"wv{eh}")
                  for eh in range(2)]
            for eh in range(2):
                nc.sync.dma_start(
                    wv[eh][:], wqkv_d[:, :, 2 * E + 512 * eh:2 * E + 512 * (eh + 1)])

            # ============ stage A: qkv projections (fp16) ============
            for n in range(4):
                xt_n = xts.tile([128, 8, 512], F16, tag="xtn")
                nc.sync.dma_start(xt_n[:], xt_d[:, :, 512 * n:512 * (n + 1)])
                # kT m-tiles for this s-block
                for m in range(8):
                    ps = ps_pool.tile([128, 512], F32, tag="mm")
                    for ct in range(8):
                        nc.tensor.matmul(ps[:], wk[m][:, ct, :], xt_n[:, ct, :],
                                         start=(ct == 0), stop=(ct == 7))
                    nc.scalar.activation(kT[:, m, 512 * n:512 * (n + 1)], ps[:],
                                         AF.Identity, bias=bqkv_col[:, 8 + m:9 + m])
                # qT (q-chunk = first block of permuted keys)
                if n == 0:
                    for m in range(8):
                        wq = wqs.tile([128, 8, 128], F16, tag="wq")
                        nc.sync.dma_start(
                            wq[:], wqkv_d[:, :, 128 * m:128 * (m + 1)])
                        ps = ps_pool.tile([128, 512], F32, tag="mm")
                        for ct in range(8):
                            nc.tensor.matmul(ps[:], wq[:, ct, :], xt_n[:, ct, :],
                                             start=(ct == 0), stop=(ct == 7))
                        nc.scalar.activation(qT[:, m, :], ps[:], AF.Identity,
                                             bias=bqkv_col[:, m:m + 1])
                # v natural rows in this block
                for sl in range(4):
                    st = 4 * n + sl
                    for eh in range(2):
                        ps = ps_pool.tile([128, 512], F32, tag="mm")
                        for ct in range(8):
                            nc.tensor.matmul(
                                ps[:], xt_n[:, ct, 128 * sl:128 * (sl + 1)],
                                wv[eh][:, ct, :], start=(ct == 0), stop=False)
                        bv = brow.tile([1, 512], F16, tag="brow16")
                        nc.sync.dma_start(bv[:],
                                          bv_row16_d[:, 512 * eh:512 * (eh + 1)])
                        nc.tensor.matmul(ps[:], ones16[:], bv[:],
                                         start=False, stop=True)
                        nc.vector.tensor_copy(vsb[:, st, 512 * eh:512 * (eh + 1)],
                                              ps[:])

            # ---- cached_kv outputs straight from the fp16 tiles ----
            nc.sync.dma_start(k_d[:], kT[:, :, 0:CH])
            nc.sync.dma_start(v_d[:], vsb[:, 0:NQT, :])

            # ============ stage B: attention ============
            # suffix-sum matmuls first (uses the "mm" psum ring, then released)
            st_ps = ps_pool.tile([128, 8, NQT], F32, tag="mm")
            for dt in range(8):
                for kt in range(NT):
                    nc.tensor.matmul(
                        st_ps[:, dt, :], vsb[:, kt, 128 * dt:128 * (dt + 1)],
                        sel[:, kt, :], start=(dt == 0 and kt == 0),
                        stop=(kt == NT - 1))
            nc.vector.tensor_copy(st_sb[:], st_ps[:])

            # two sequential q-halves of 256 (bounds PSUM: yps 8KB + mm ring 8KB)
            for hf in range(2):
                q0 = 256 * hf
                yps = ps_y.tile([128, 8, 256], F32, tag="y", name=f"yps{hf}")
                for kt in range(NT):
                    e_t = epool.tile([128, H, 256], F16, tag="E")
                    for h in range(H):
                        po = 64 * (h % 2)
                        sc = ps_pool.tile([128, 256], F32, tag="mm")
                        nc.tensor.matmul(
                            sc[:], kT[po:po + 64, h // 2, 128 * kt:128 * (kt + 1)],
                            qT[po:po + 64, h // 2, q0:q0 + 256],
                            start=True, stop=True)
                        nc.scalar.activation(e_t[:, h, :], sc[:], AF.Exp)
                    # denominator tree (fp16, DVE 2x)
                    t8 = [dpool.tile([128, 256], F16, tag=f"d8_{i%4}",
                                     name=f"t8_{i}") for i in range(8)]
                    for i in range(8):
                        nc.vector.tensor_add(t8[i][:], e_t[:, 2 * i, :],
                                             e_t[:, 2 * i + 1, :])
                    t4 = [dpool.tile([128, 256], F16, tag=f"d4_{i%2}",
                                     name=f"t4_{i}") for i in range(4)]
                    for i in range(4):
                        nc.vector.tensor_add(t4[i][:], t8[2 * i][:], t8[2 * i + 1][:])
                    t2 = [dpool.tile([128, 256], F16, tag="d2", name=f"t2_{i}")
                          for i in range(2)]
                    for i in range(2):
                        nc.vector.tensor_add(t2[i][:], t4[2 * i][:], t4[2 * i + 1][:])
                    dd32 = dpool.tile([128, 256], F32, tag="dd32")
                    nc.vector.tensor_add(dd32[:], t2[0][:], t2[1][:])
                    rr32 = dpool.tile([128, 256], F32, tag="rr32")
                    nc.vector.reciprocal_approx_fast(rr32[:], dd32[:])
                    rr = dpool.tile([128, 256], F16, tag="rr")
                    nc.vector.tensor_copy(rr[:], rr32[:])
                    rg = dpool.tile([128, 256], F16, tag="rg")
                    for jj in range(2):
                        j = 2 * hf + jj
                        nc.vector.tensor_scalar_mul(
                            rg[:, 128 * jj:128 * (jj + 1)],
                            rr[:, 128 * jj:128 * (jj + 1)],
                            agate[:, kt, j:j + 1])
                    att = apool.tile([128, H, 256], F16, tag="att")
                    for h in range(H):
                        nc.vector.tensor_mul(att[:, h, :], e_t[:, h, :], rg[:])
                    for h in range(H):
                        po = 64 * (h % 2)
                        nc.tensor.matmul(
                            yps[po:po + 64, h // 2, :],
                            vsb[:, kt, 64 * h:64 * (h + 1)],
                            att[:, h, :],
                            start=(kt == 0 and h % 4 < 2),
                            stop=(kt == NT - 1))
                    # diagonal tile: exact softmax w/ intra-tile causal fill 1/16
                    if kt < NQT and kt // 2 == hf:
                        j = kt
                        jj = j % 2
                        datt = dpool.tile([128, H, 128], F16, tag="datt", bufs=1)
                        for h in range(H):
                            nc.vector.tensor_mul(
                                datt[:, h, :], e_t[:, h, 128 * jj:128 * (jj + 1)],
                                rr[:, 128 * jj:128 * (jj + 1)])
                            nc.gpsimd.affine_select(
                                out=datt[:, h, :], in_=datt[:, h, :],
                                compare_op=ALU.is_ge, fill=0.0625, base=0,
                                pattern=[[1, 128]], channel_multiplier=-1)
                        for h in range(H):
                            po = 64 * (h % 2)
                            nc.tensor.matmul(
                                yps[po:po + 64, h // 2,
                                    128 * jj:128 * (jj + 1)],
                                vsb[:, kt, 64 * h:64 * (h + 1)],
                                datt[:, h, :], start=False, stop=False)
                # finalize this half: yT = yps + suffix (per-partition scalar)
                for pair in range(8):
                    for jj in range(2):
                        j = 2 * hf + jj
                        nc.vector.tensor_scalar_add(
                            yT[:, pair, 128 * j:128 * (j + 1)],
                            yps[:, pair, 128 * jj:128 * (jj + 1)],
                            st_sb[:, pair, j:j + 1])

            # ============ stage C: dense ============
            for eh in range(2):
                wd_t = epool.tile([128, 8, 512], F16, tag="E", name=f"wd{eh}")
                nc.sync.dma_start(wd_t[:], wd_d[:, :, 512 * eh:512 * (eh + 1)])
                bd = brow.tile([1, 512], F16, tag="brow16")
                nc.sync.dma_start(bd[:], bd_row_d[:, 512 * eh:512 * (eh + 1)])
                for qt in range(NQT):
                    ps = ps_pool.tile([128, 512], F32, tag="mm")
                    for ct in range(8):
                        nc.tensor.matmul(
                            ps[:], yT[:, ct, 128 * qt:128 * (qt + 1)],
                            wd_t[:, ct, :], start=(ct == 0), stop=False)
                    nc.tensor.matmul(ps[:], ones16[:], bd[:],
                                     start=False, stop=True)
                    y_sb = stage.tile([128, 512], F32, tag="stg")
                    nc.vector.tensor_copy(y_sb[:], ps[:])
                    nc.sync.dma_start(
                        y_d[128 * qt:128 * (qt + 1), 512 * eh:512 * (eh + 1)],
                        y_sb[:])

    nc.compile()
    return nc


def _prep_core_inputs(x, W_qkv, b_qkv, W_dense, b_dense, core):
    b, c = core % 2, core // 2
    own = np.arange(CH * c, CH * (c + 1))
    rest = np.concatenate([np.arange(0, CH * c), np.arange(CH * (c + 1), S)])
    perm = np.concatenate([own, rest])
    orig_tile = np.concatenate([np.arange(4 * c, 4 * c + 4),
                                np.array([t for t in range(NT)
                                          if not (4 * c <= t < 4 * c + 4)])])

    xb = x[b]                                    # [S, E]
    xt = np.ascontiguousarray(xb[perm].T)        # [E, S]
    xt16 = xt.reshape(8, 128, S).transpose(1, 0, 2).astype(np.float16)

    wq = (W_qkv[:, :E] / 8.0)
    wqkv_mod = np.concatenate([wq, W_qkv[:, E:]], axis=1)
    wqkv16 = wqkv_mod.reshape(8, 128, 3 * E).transpose(1, 0, 2).astype(np.float16)
    wd16 = W_dense.reshape(8, 128, E).transpose(1, 0, 2).astype(np.float16)

    bq = b_qkv[:E] / 8.0
    bqkv_col = np.concatenate([bq, b_qkv[E:2 * E]]).reshape(16, 128).T.astype(np.float32)
    bv_row16 = b_qkv[2 * E:].reshape(1, E).astype(np.float16)
    bd_row = b_dense.reshape(1, E).astype(np.float16)

    qt_g = 4 * c + np.arange(NQT)                # global q-tile indices
    ot = orig_tile[:, None]                      # [NT, 1]
    a = (ot < qt_g[None, :]).astype(np.float32)  # full-attention gate
    s = np.where(ot > qt_g[None, :], 1.0 / 16.0, 0.0).astype(np.float32)
    agate = np.broadcast_to(a, (128, NT, NQT)).astype(np.float32)
    sel = np.broadcast_to(s, (128, NT, NQT)).astype(np.float16)

    return dict(xt=np.ascontiguousarray(xt16), wqkv=np.ascontiguousarray(wqkv16),
                wd=np.ascontiguousarray(wd16), bqkv_col=bqkv_col,
                bv_row16=bv_row16, bd_row=bd_row,
                agate=np.ascontiguousarray(agate), sel=np.ascontiguousarray(sel))


def kernel(x, W_qkv, b_qkv, W_dense, b_dense):
    global LAST_RESULTS, _CACHED_NC
    x = np.asarray(x, np.float32)
    W_qkv = np.asarray(W_qkv, np.float32)
    b_qkv = np.asarray(b_qkv, np.float32)
    W_dense = np.asarray(W_dense, np.float32)
    b_dense = np.asarray(b_dense, np.float32)

    if _CACHED_NC is None:
        _CACHED_NC = _build_nc()
    nc = _CACHED_NC

    in_maps = [_prep_core_inputs(x, W_qkv, b_qkv, W_dense, b_dense, core)
               for core in range(NCORES)]
    res = run_bass_kernel_spmd(nc, in_maps, core_ids=list(range(NCORES)))
    LAST_RESULTS = res

    y = np.zeros((B, S, E), np.float32)
    kv = np.zeros((B, 2, H, S, D), np.float32)
    for core in range(NCORES):
        b, c = core % 2, core // 2
        r = res.results[core]
        y[b, CH * c:CH * (c + 1)] = r["y_out"]
        # k_out [128, 8, CH] fp16: k_nat[s, 128m+p] = k_out[p, m, s]
        k_nat = r["k_out"].astype(np.float32).transpose(1, 0, 2).reshape(E, CH).T
        v_nat = r["v_out"].astype(np.float32).transpose(1, 0, 2).reshape(CH, E)
        kv[b, 0, :, CH * c:CH * (c + 1), :] = \
            k_nat.reshape(CH, H, D).transpose(1, 0, 2)
        kv[b, 1, :, CH * c:CH * (c + 1), :] = \
            v_nat.reshape(CH, H, D).transpose(1, 0, 2)
    return y, kv
